# revision 19
# baseline (speedup 1.0000x reference)
"""Trainium2 Bass kernel for nn_BasicLayer (sparse cluster attention, 2 layers).

v2 design
---------
Scanline order commutes with per-token ops, so gather to curve order on host,
run both layers on-device over contiguous 64-token clusters, scatter back.
8192 tokens/core (half a batch), weights replicated.

Per-core pipeline (TILE=1024 tokens = 8 subs of 128 = 16 cluster pairs):
- All layout flips (LN token-major -> feature-major, P -> P^T) go through the
  DMA crossbar (dma_start_transpose, batched), not the PE. No identity matmuls.
- q/k stored as 3-head groups [96, T] so per-head score matmuls slice at
  partition offsets {0,32,64} directly (offset 96 is unencodable).
- Scalar activation-table switches (Sqrt/Exp/Gelu) are prefetched with dummy
  ops so the ~1.5us ACT_TABLE_LOAD never sits on the critical path.
- LN chains (bn_stats/sqrt/recip/normalize/xbar) are emitted interleaved with
  the previous phase's residual adds, per 4-sub half, so the PE never waits
  on a cold LN chain at a phase boundary.
- Cross-cluster softmax leakage is killed by persistent zero off-diagonal
  blocks in P (normalize writes diagonal blocks only).
"""

import os
import numpy as np
import ml_dtypes

# ---- problem constants (hardcoded per contract) ----
B, N, D = 4, 16384, 192
HEADS, DH, CLM = 6, 32, 64
GRID_W = 128
DEPTH = 2
NCORES = 8
T = (B * N) // NCORES                # 8192 tokens per core
SUB = 128
NSUB = 8
TILE = SUB * NSUB                    # 1024-token supertile
NTILES = T // TILE                   # 8
DFF = 768

_STOP_PHASE = None   # debug: "qk"|"scores"|"attn"|None
_COMPILED = {}


def _scanline_order(pos, w):
    ix = np.floor(pos[..., 0]).astype(np.int64)
    iy = np.floor(pos[..., 1]).astype(np.int64)
    key = iy * w + np.where(iy % 2 == 1, w - 1 - ix, ix)
    return np.argsort(key, axis=1, kind="stable")


def _fold_weights(inputs):
    """Fold LN affine + biases into matmul weights. Returns per-layer dicts
    laid out exactly as the DRAM tensors the kernel declares."""
    bf16 = ml_dtypes.bfloat16
    scale = DH ** -0.5
    layers = []
    for i in range(DEPTH):
        g1 = np.asarray(inputs["ln1_g"][i], np.float64)
        b1 = np.asarray(inputs["ln1_b"][i], np.float64)
        Wqkv = np.asarray(inputs["w_qkv"][i], np.float64)
        bqkv = np.asarray(inputs["b_qkv"][i], np.float64)
        w_eff = g1[:, None] * Wqkv
        b_eff = b1 @ Wqkv + bqkv
        wq = w_eff[:, 0:D] * scale
        bq = b_eff[0:D] * scale
        wk = w_eff[:, D:2 * D]
        bk = b_eff[D:2 * D]
        wv = w_eff[:, 2 * D:3 * D]
        bv = b_eff[2 * D:3 * D]
        # qk weight M-layout: 3-head groups [q h0-2 | k h0-2 | q h3-5 | k h3-5]
        wqk = np.concatenate(
            [wq[:, :96], wk[:, :96], wq[:, 96:], wk[:, 96:]], axis=1)
        bqk = np.stack([bq[:96], bk[:96], bq[96:], bk[96:]], axis=1)
        wp = np.asarray(inputs["w_proj"][i], np.float64)
        bp = np.asarray(inputs["b_proj"][i], np.float64)
        g2 = np.asarray(inputs["ln2_g"][i], np.float64)
        b2 = np.asarray(inputs["ln2_b"][i], np.float64)
        W1 = np.asarray(inputs["w_fc1"][i], np.float64)
        w1_eff = g2[:, None] * W1
        b1_eff = b2 @ W1 + np.asarray(inputs["b_fc1"][i], np.float64)
        W2 = np.asarray(inputs["w_fc2"][i], np.float64)
        bfc2 = np.asarray(inputs["b_fc2"][i], np.float64)
        bv_t = np.stack([bv[:96], bv[96:]], axis=1)
        layers.append({
            f"wqk{i}": wqk.astype(bf16),
            f"bqk{i}": bqk.astype(np.float32),
            f"wv{i}": wv.astype(bf16),
            f"bv{i}": bv_t.astype(np.float32),
            f"wp{i}": wp.astype(bf16),
            f"bp{i}": np.tile(bp.astype(np.float32), (128, 1)),
            f"w1{i}": w1_eff.astype(bf16),
            f"b1{i}": b1_eff.reshape(6, 128).T.copy().astype(np.float32),
            f"w2{i}": W2.astype(bf16),
            f"b2{i}": np.tile(bfc2.astype(np.float32), (128, 1)),
        })
    return layers


def _build_nc(biases_zero=False, ntiles=NTILES):
    key = ("nc", biases_zero, ntiles)
    if key in _COMPILED:
        return _COMPILED[key]

    from contextlib import ExitStack
    import concourse.bass as bass
    import concourse.tile as tile
    from concourse import bacc, mybir
    from concourse.bass import ts, ds

    f32 = mybir.dt.float32
    bf16 = mybir.dt.bfloat16
    AF = mybir.ActivationFunctionType
    OP = mybir.AluOpType

    nc = bacc.Bacc("TRN2", target_bir_lowering=False, debug=False,
                   enable_asserts=False, num_devices=NCORES)

    x_d = nc.dram_tensor("x", [T, D], f32, kind="ExternalInput").ap()
    y_d = nc.dram_tensor("y", [T, D], f32, kind="ExternalOutput").ap()
    wd = []
    for i in range(DEPTH):
        wd.append({
            "wqk": nc.dram_tensor(f"wqk{i}", [D, 384], bf16, kind="ExternalInput").ap(),
            "bqk": nc.dram_tensor(f"bqk{i}", [96, 4], f32, kind="ExternalInput").ap(),
            "wv": nc.dram_tensor(f"wv{i}", [D, D], bf16, kind="ExternalInput").ap(),
            "bv": nc.dram_tensor(f"bv{i}", [96, 2], f32, kind="ExternalInput").ap(),
            "wp": nc.dram_tensor(f"wp{i}", [D, D], bf16, kind="ExternalInput").ap(),
            "bp": nc.dram_tensor(f"bp{i}", [128, D], f32, kind="ExternalInput").ap(),
            "w1": nc.dram_tensor(f"w1{i}", [D, DFF], bf16, kind="ExternalInput").ap(),
            "b1": nc.dram_tensor(f"b1{i}", [128, 6], f32, kind="ExternalInput").ap(),
            "w2": nc.dram_tensor(f"w2{i}", [DFF, D], bf16, kind="ExternalInput").ap(),
            "b2": nc.dram_tensor(f"b2{i}", [128, D], f32, kind="ExternalInput").ap(),
        })

    def bcast(ap2d, n):
        return bass.AP(tensor=ap2d.tensor, offset=ap2d.offset,
                       ap=[*ap2d.ap, [0, n]])

    with tile.TileContext(nc) as tc, ExitStack() as ctx:
        consts = ctx.enter_context(tc.tile_pool(name="consts", bufs=1))
        xpool = ctx.enter_context(tc.tile_pool(name="xpool", bufs=3))
        lnpool = ctx.enter_context(tc.tile_pool(name="lnpool", bufs=3))
        qkpool = ctx.enter_context(tc.tile_pool(name="qkpool", bufs=2))
        apool = ctx.enter_context(tc.tile_pool(name="apool", bufs=3))
        mpool = ctx.enter_context(tc.tile_pool(name="mpool", bufs=2))
        stpool = ctx.enter_context(tc.tile_pool(name="stpool", bufs=8))
        pp_sc = ctx.enter_context(tc.tile_pool(name="pp_sc", bufs=6, space="PSUM"))
        pp_x = ctx.enter_context(tc.tile_pool(name="pp_x", bufs=2, space="PSUM"))

        eps_t = consts.tile([128, 1], f32)
        nc.vector.memset(eps_t, 1e-5)
        u32 = mybir.dt.uint32
        rsq_c1 = consts.tile([128, 1], u32, name="rsq_c1")
        nc.vector.memset(rsq_c1, 1)
        rsq_cm = consts.tile([128, 1], u32, name="rsq_cm")
        nc.vector.memset(rsq_cm, 0x5F3759DF)
        dsrc = consts.tile([128, 1], f32)
        nc.vector.memset(dsrc, 0.5)
        ddst = consts.tile([128, 1], f32, name="ddst")

        def prefetch(af):
            nc.scalar.activation(ddst, dsrc, af)

        # persistent softmax tiles: off-diagonal (cross-cluster) blocks stay 0
        NPBUF = 4
        p_bufs = []
        for pb_i in range(NPBUF):
            pb = consts.tile([128, 2, HEADS, 128], bf16, name=f"pbuf{pb_i}")
            nc.vector.memset(pb, 0.0)
            p_bufs.append(pb)
        pctr = [0]

        # --- weights to SBUF once ---
        W = []
        for i in range(DEPTH):
            d = wd[i]
            sb = {}
            sb["wqkA"] = consts.tile([128, 384], bf16, name=f"wqkA{i}")
            sb["wqkB"] = consts.tile([128, 384], bf16, name=f"wqkB{i}")
            nc.sync.dma_start(out=sb["wqkA"], in_=d["wqk"][0:128])
            nc.sync.dma_start(out=sb["wqkB"][ds(64, 64)], in_=d["wqk"][128:192])
            sb["wvA"] = consts.tile([128, D], bf16, name=f"wvA{i}")
            sb["wvB"] = consts.tile([128, D], bf16, name=f"wvB{i}")
            nc.sync.dma_start(out=sb["wvA"], in_=d["wv"][0:128])
            nc.sync.dma_start(out=sb["wvB"][ds(64, 64)], in_=d["wv"][128:192])
            sb["wpA"] = consts.tile([96, D], bf16, name=f"wpA{i}")
            sb["wpB"] = consts.tile([96, D], bf16, name=f"wpB{i}")
            nc.sync.dma_start(out=sb["wpA"], in_=d["wp"][0:96])
            nc.sync.dma_start(out=sb["wpB"], in_=d["wp"][96:192])
            sb["w1A"] = consts.tile([128, DFF], bf16, name=f"w1A{i}")
            sb["w1B"] = consts.tile([128, DFF], bf16, name=f"w1B{i}")
            nc.sync.dma_start(out=sb["w1A"], in_=d["w1"][0:128])
            nc.sync.dma_start(out=sb["w1B"][ds(64, 64)], in_=d["w1"][128:192])
            sb["w2m"] = consts.tile([128, 6, D], bf16, name=f"w2m{i}")
            nc.sync.dma_start(
                out=sb["w2m"],
                in_=d["w2"].rearrange("(m p) n -> p m n", p=128))
            for nm, shp in (("bqk", [96, 4]), ("bv", [96, 2]), ("b1", [128, 6]),
                            ("bp", [128, D]), ("b2", [128, D])):
                sb[nm] = consts.tile(shp, f32, name=f"{nm}{i}")
                nc.sync.dma_start(out=sb[nm], in_=d[nm])
            W.append(sb)

        # ---------- LN chain helpers ----------
        def ln_tiles():
            return {
                "mv": lnpool.tile([128, NSUB, 2], f32, tag="mv", name="mv"),
                "sd": lnpool.tile([128, NSUB], f32, tag="sd", name="sd"),
                "r": lnpool.tile([128, NSUB], f32, tag="r", name="r"),
                "xnA": lnpool.tile([128, NSUB, 128], bf16, tag="xnA", name="xnA"),
                "xnB": lnpool.tile([128, NSUB, 128], bf16, tag="xnB", name="xnB"),
                "fmA": lnpool.tile([128, NSUB, 128], bf16, tag="fmA", name="fmA"),
                "fmB": lnpool.tile([128, NSUB, 128], bf16, tag="fmB", name="fmB"),
            }

        def ln_stats(ln, x_t, s):
            st = stpool.tile([128, 6], f32, tag="st", name="st")
            nc.vector.bn_stats(st, x_t[:, s, 0:D])
            nc.vector.bn_aggr(ln["mv"][:, s], st)

        def ln_finish_half(ln, x_t, h):
            # r = 1/sqrt(var) on DVE: magic-constant seed + 2 Newton steps
            # (~5e-6 rel err; the 1e-5 eps is dropped, negligible at var~1).
            # Keeps Sqrt off the scalar engine so the Exp/Gelu activation
            # tables never get evicted mid-phase.
            s0 = 4 * h
            var = ln["mv"][:, s0:s0 + 4, 1]
            t1 = ln["sd"][:, s0:s0 + 4]
            yv = stpool.tile([128, 4], f32, tag="ya", name="ya")
            av = stpool.tile([128, 4], f32, tag="av", name="av")
            rr4 = ln["r"][:, s0:s0 + 4]
            nc.vector.tensor_tensor(out=t1.bitcast(u32), in0=var.bitcast(u32),
                                    in1=bcast(rsq_c1, 4), op=OP.logical_shift_right)
            nc.vector.tensor_tensor(out=yv.bitcast(u32), in0=bcast(rsq_cm, 4),
                                    in1=t1.bitcast(u32), op=OP.subtract)
            cur = yv
            for it in range(2):
                nxt = rr4 if it == 1 else t1
                nc.vector.tensor_tensor(out=av, in0=var, in1=cur, op=OP.mult)
                nc.vector.tensor_tensor(out=av, in0=av, in1=cur, op=OP.mult)
                nc.vector.tensor_scalar(out=av, in0=av, scalar1=-0.5,
                                        scalar2=1.5, op0=OP.mult, op1=OP.add)
                nc.vector.tensor_tensor(out=nxt, in0=cur, in1=av, op=OP.mult)
                cur = nxt
            for s in range(s0, s0 + 4):
                mu = ln["mv"][:, s, 0:1]
                rr = ln["r"][:, s:s + 1]
                mu_b = bcast(mu, 128)
                r_b = bcast(rr, 128)
                tA = stpool.tile([128, 128], bf16, tag="tA", name="tA")
                nc.gpsimd.tensor_tensor(out=tA, in0=x_t[:, s, 0:128],
                                        in1=mu_b, op=OP.subtract)
                nc.gpsimd.tensor_tensor(out=ln["xnA"][:, s], in0=tA,
                                        in1=r_b, op=OP.mult)
                tB = stpool.tile([128, 128], bf16, tag="tB", name="tB")
                eng = nc.vector if s % 2 == 0 else nc.gpsimd
                eng.tensor_tensor(out=tB, in0=x_t[:, s, 64:192],
                                  in1=mu_b, op=OP.subtract)
                eng.tensor_tensor(out=ln["xnB"][:, s], in0=tB,
                                  in1=r_b, op=OP.mult)
            nc.sync.dma_start_transpose(out=ln["fmA"][:, s0:s0 + 4, :],
                                        in_=ln["xnA"][:, s0:s0 + 4, :])
            nc.sync.dma_start_transpose(out=ln["fmB"][:, s0:s0 + 4, :],
                                        in_=ln["xnB"][:, s0:s0 + 4, :])

        def fmA_half(ln, h):
            return ln["fmA"][:, 4 * h:4 * h + 4, :].rearrange("p a b -> p (a b)")

        def fmB_half(ln, h):
            return ln["fmB"][ds(64, 64), 4 * h:4 * h + 4, :].rearrange(
                "p a b -> p (a b)")

        # ======================================================================
        def emit_layer(sb, x_t, ln1, x_nxt, nxt_is_tile):
            """Emit one transformer layer. ln1 holds this layer's LN1 (already
            emitted). Returns the LN tiles for the NEXT layer's LN1 (computed
            on x_nxt: either this x_t post-MLP, or the next tile's x)."""
            # ---- qk: 4 blocks x 2 halves ----
            qA = qkpool.tile([96, TILE], bf16, tag="qA", name="qA")
            kA = qkpool.tile([96, TILE], bf16, tag="kA", name="kA")
            qB = qkpool.tile([96, TILE], bf16, tag="qB", name="qB")
            kB = qkpool.tile([96, TILE], bf16, tag="kB", name="kB")
            dsts = (qA, kA, qB, kB)
            for hf in range(2):
                rA, rB = fmA_half(ln1, hf), fmB_half(ln1, hf)
                for m in range(4):
                    ps = pp_x.tile([96, 512], f32, tag="x", name="psqk")
                    nc.tensor.matmul(ps, sb["wqkA"][:, ds(96 * m, 96)], rA,
                                     start=True, stop=False)
                    nc.tensor.matmul(ps, sb["wqkB"][ds(64, 64), ds(96 * m, 96)], rB,
                                     start=False, stop=True)
                    dst = dsts[m][:, ts(hf, 512)]
                    if not biases_zero:
                        nc.scalar.activation(dst, ps, AF.Identity,
                                             bias=sb["bqk"][:, m:m + 1])
                    else:
                        nc.vector.tensor_copy(dst, ps)

            if _STOP_PHASE == "qk":
                return ln1
            # ---- scores + softmax + v, per sub ----
            v_tm = apool.tile([128, NSUB, D], bf16, tag="vtm", name="v_tm")
            pkms = []
            for s in range(NSUB):
                # one PSUM tile per PE tile-row: heads (hh, hh+3) share row
                # 32*hh; independent accumulation groups at different rows in
                # one PSUM tile lock up the device
                scR = [pp_sc.tile([128, 2, 128], f32, tag="sc", name=f"scR{r}")
                       for r in range(3)]
                for hd in range(HEADS):
                    grp, hh = hd // 3, hd % 3
                    qsrc = (qA, qB)[grp]
                    ksrc = (kA, kB)[grp]
                    nc.tensor.matmul(scR[hh][:, grp],
                                     qsrc[ds(32 * hh, 32), ts(s, 128)],
                                     ksrc[ds(32 * hh, 32), ts(s, 128)],
                                     start=True, stop=True)
                if _STOP_PHASE == "sconly":
                    continue
                # v for this sub (PE filler between score groups)
                psv = pp_x.tile([128, D], f32, tag="x", name="psv")
                nc.tensor.matmul(psv, ln1["fmA"][:, s, :], sb["wvA"],
                                 start=True, stop=False)
                nc.tensor.matmul(psv, ln1["fmB"][ds(64, 64), s, :], sb["wvB"][ds(64, 64)],
                                 start=False, stop=True)
                nc.vector.tensor_copy(v_tm[:, s], psv)
                if _STOP_PHASE == "sv":
                    continue
                # softmax (diagonal 64-blocks only)
                E = apool.tile([128, HEADS, 128], bf16, tag="E", name="E")
                sums = stpool.tile([128, HEADS], f32, tag="sm", name="sums")
                rsum = stpool.tile([128, HEADS], f32, tag="rs", name="rsum")
                for hh in range(3):
                    for half in range(2):
                        ho = half * 64
                        ebase = E[ds(ho, 64)]
                        esl = bass.AP(
                            tensor=ebase.tensor,
                            offset=ebase.offset + hh * 128 + ho,
                            ap=[ebase.ap[0], [3 * 128, 2], [1, 64]])
                        nc.scalar.activation(
                            esl, scR[hh][ds(ho, 64), :, ho:ho + 64], AF.Exp)
                if _STOP_PHASE == "exp":
                    continue
                nc.vector.reduce_sum(sums[0:64], E[0:64, :, 0:64],
                                     axis=mybir.AxisListType.X)
                nc.vector.reduce_sum(sums[ds(64, 64)], E[ds(64, 64), :, 64:128],
                                     axis=mybir.AxisListType.X)
                nc.vector.reciprocal(rsum, sums)
                if _STOP_PHASE == "sums":
                    continue
                P = p_bufs[(pctr[0] // 2) % NPBUF]
                sp = pctr[0] % 2
                pctr[0] += 1
                for half in range(2):
                    hs = ds(half * 64, 64)
                    rs_half = rsum[ds(half * 64, 64)]
                    nc.gpsimd.tensor_tensor(
                        out=P[hs, sp, :, hs], in0=E[hs, :, hs],
                        in1=bcast(rs_half, 64), op=OP.mult)
                if _STOP_PHASE == "norm":
                    continue
                if sp == 1:
                    pkm = apool.tile([128, 2 * HEADS, 128], bf16, tag="pkm",
                                     name="pkm")
                    nc.sync.dma_start_transpose(
                        out=pkm, in_=P.rearrange("p a b c -> p (a b c)"))
                    pkms.append(pkm)

            if _STOP_PHASE in ("sconly", "sconly2", "sv", "exp", "sums", "norm", "scores"):
                return ln1
            # next-tile LN1 depends only on the next x DMA: emit its whole
            # chain here so it overlaps this layer's attention tail + MLP
            ln_early = None
            if nxt_is_tile and x_nxt is not None:
                ln_early = ln_tiles()
                for s2 in range(NSUB):
                    ln_stats(ln_early, x_nxt, s2)
                ln_finish_half(ln_early, x_nxt, 0)
                ln_finish_half(ln_early, x_nxt, 1)

            # ---- O + out-copies + proj + residual + LN2 stats, per sub ----
            ln2 = ln_tiles()
            ofm0 = apool.tile([96, TILE], bf16, tag="ofm0", name="ofm0")
            ofm1 = apool.tile([96, TILE], bf16, tag="ofm1", name="ofm1")
            for s in range(NSUB):
                oP = pp_x.tile([96, 256], f32, tag="x", name="oP")
                for hd in range(HEADS):
                    grp, hh = hd // 3, hd % 3
                    nc.tensor.matmul(oP[ds(32 * hh, 32), ts(grp, 128)],
                                     v_tm[:, s, ts(hd, 32)],
                                     pkms[s // 2][:, 6 * (s % 2) + hd],
                                     start=True, stop=True,
                                     tile_position=(0, 32 * hh))
                if not biases_zero:
                    nc.scalar.activation(ofm0[:, ts(s, 128)], oP[:, 0:128],
                                         AF.Identity, bias=sb["bv"][:, 0:1])
                    nc.scalar.activation(ofm1[:, ts(s, 128)], oP[:, 128:256],
                                         AF.Identity, bias=sb["bv"][:, 1:2])
                else:
                    nc.vector.tensor_copy(ofm0[:, ts(s, 128)], oP[:, 0:128])
                    nc.vector.tensor_copy(ofm1[:, ts(s, 128)], oP[:, 128:256])
                psp = pp_x.tile([128, D], f32, tag="x", name="psp")
                nc.tensor.matmul(psp, ofm0[:, ts(s, 128)], sb["wpA"],
                                 start=True, stop=False)
                nc.tensor.matmul(psp, ofm1[:, ts(s, 128)], sb["wpB"],
                                 start=False, stop=True)
                nc.vector.tensor_add(x_t[:, s, 0:D], x_t[:, s, 0:D], psp)
                if not biases_zero:
                    nc.vector.tensor_add(x_t[:, s, 0:D], x_t[:, s, 0:D],
                                         sb["bp"])
                ln_stats(ln2, x_t, s)
                if s == 3:
                    ln_finish_half(ln2, x_t, 0)
                elif s == 7:
                    ln_finish_half(ln2, x_t, 1)
                    prefetch(AF.Gelu_apprx_tanh)

            if _STOP_PHASE == "attn":
                return ln2
            # ---- MLP: fc1+gelu both halves, then fc2 + residual ----
            hfms = []
            for hf in range(2):
                rA, rB = fmA_half(ln2, hf), fmB_half(ln2, hf)
                hfm = mpool.tile([128, 6, 512], bf16, tag="hfm", name="hfm")
                for m in range(6):
                    ps = pp_x.tile([128, 512], f32, tag="x", name="psh")
                    nc.tensor.matmul(ps, sb["w1A"][:, ts(m, 128)], rA,
                                     start=True, stop=False)
                    nc.tensor.matmul(ps, sb["w1B"][ds(64, 64), ts(m, 128)], rB,
                                     start=False, stop=True)
                    if biases_zero:
                        nc.scalar.activation(hfm[:, m], ps, AF.Gelu_apprx_tanh)
                    else:
                        nc.scalar.activation(hfm[:, m], ps, AF.Gelu_apprx_tanh,
                                             bias=sb["b1"][:, m:m + 1])
                hfms.append(hfm)
            ln_n = ln_early if (nxt_is_tile and ln_early is not None) else ln_tiles()
            for hf in range(2):
                for si in range(4):
                    s = 4 * hf + si
                    ps = pp_x.tile([128, D], f32, tag="x", name="psf2")
                    for m in range(6):
                        nc.tensor.matmul(ps, hfms[hf][:, m, ts(si, 128)],
                                         sb["w2m"][:, m],
                                         start=(m == 0), stop=(m == 5))
                    nc.vector.tensor_add(x_t[:, s, 0:D], x_t[:, s, 0:D], ps)
                    if not biases_zero:
                        nc.vector.tensor_add(x_t[:, s, 0:D], x_t[:, s, 0:D],
                                             sb["b2"])
                    if x_nxt is None or nxt_is_tile:
                        continue
                    ln_stats(ln_n, x_t, s)
                if x_nxt is None or nxt_is_tile:
                    continue
                if hf == 0:
                    ln_finish_half(ln_n, x_t, 0)
                else:
                    ln_finish_half(ln_n, x_t, 1)
                    prefetch(AF.Exp)
            if nxt_is_tile:
                prefetch(AF.Exp)
            return ln_n

        # ======================================================================
        def load_tile(it):
            xt = xpool.tile([128, NSUB, D], f32, tag="x", name="x_t")
            nc.sync.dma_start(
                out=xt, in_=x_d[ts(it, TILE)].rearrange("(s p) f -> p s f",
                                                        p=128))
            return xt

        x_cur = load_tile(0)
        ln = ln_tiles()
        for s in range(4):
            ln_stats(ln, x_cur, s)
        ln_finish_half(ln, x_cur, 0)
        for s in range(4, 8):
            ln_stats(ln, x_cur, s)
        ln_finish_half(ln, x_cur, 1)
        prefetch(AF.Exp)

        for it in range(ntiles):
            x_nxt = load_tile(it + 1) if it + 1 < ntiles else None
            # layer 0: next LN is layer 1's LN1 on this x_t
            ln = emit_layer(W[0], x_cur, ln, x_cur, nxt_is_tile=False)
            # layer 1: next LN is the next tile's LN1 on x_nxt
            ln = emit_layer(W[1], x_cur, ln, x_nxt, nxt_is_tile=True)
            nc.sync.dma_start(
                out=y_d[ts(it, TILE)].rearrange("(s p) f -> p s f", p=128),
                in_=x_cur[:, :, 0:D])
            x_cur = x_nxt

    nc.compile()
    _COMPILED[key] = nc
    return nc


def _ensure_ntff_hook():
    """The image's antenv package lacks axon_hooks; synthesize it and install
    the ctypes-based NTFF profile hook from trn_agent_boot (test-only path)."""
    import sys, types
    if "antenv.axon_hooks" in sys.modules:
        return True
    try:
        mod = types.ModuleType("antenv.axon_hooks")
        state = {}
        mod.set_axon_ntff_profile_hook = lambda h: state.__setitem__("h", h)
        mod.get_axon_ntff_profile_hook = lambda: state.get("h")
        sys.modules["antenv.axon_hooks"] = mod
        import antenv
        antenv.axon_hooks = mod
        from trn_agent_boot.trn_boot import _ntff_profile_via_ctypes
        mod.set_axon_ntff_profile_hook(
            _ntff_profile_via_ctypes("/opt/axon/libaxon_pjrt.so"))
        return True
    except Exception as e:  # pragma: no cover
        print(f"NTFF hook shim failed: {e}")
        return False


def _run(inputs, trace=False):
    """Shard, execute on 8 cores, gather. Returns (y_full, exec_time_ns)."""
    from concourse.bass_utils import run_bass_kernel_spmd

    if trace:
        trace = _ensure_ntff_hook()

    layers = _fold_weights(inputs)
    bz = all(
        not np.any(np.asarray(d[k], np.float32))
        for d in layers for k in d
        if k.startswith(("bp", "b2", "bqk", "bv", "b1")))
    nc = _build_nc(biases_zero=bz)

    x = np.asarray(inputs["x"], np.float32)
    pos = np.asarray(inputs["pos"], np.float32)
    w = int(np.asarray(inputs["w"]))
    order = _scanline_order(pos, w)
    x_ord = np.take_along_axis(x, order[..., None], axis=1)
    shards = np.ascontiguousarray(x_ord.reshape(NCORES, T, D))

    wmap = {}
    for d in layers:
        wmap.update({k: np.ascontiguousarray(v) for k, v in d.items()})

    in_maps = [{"x": shards[c], **wmap} for c in range(NCORES)]
    res = run_bass_kernel_spmd(nc, in_maps, core_ids=list(range(NCORES)),
                               trace=trace)
    y_ord = np.stack([res.results[c]["y"] for c in range(NCORES)])
    y_ord = y_ord.reshape(B, N, D)
    y = np.empty_like(y_ord)
    np.put_along_axis(y, order[..., None], y_ord, axis=1)
    return y.astype(np.float32), res.exec_time_ns


def kernel(**inputs):
    y, _ = _run(inputs, trace=False)
    return y


# revision 21
# speedup vs baseline: 1.0126x; 1.0126x over previous
"""Trainium2 Bass kernel for nn_BasicLayer (sparse cluster attention, 2 layers).

v2 design
---------
Scanline order commutes with per-token ops, so gather to curve order on host,
run both layers on-device over contiguous 64-token clusters, scatter back.
8192 tokens/core (half a batch), weights replicated.

Per-core pipeline (TILE=1024 tokens = 8 subs of 128 = 16 cluster pairs):
- All layout flips (LN token-major -> feature-major, P -> P^T) go through the
  DMA crossbar (dma_start_transpose, batched), not the PE. No identity matmuls.
- q/k stored as 3-head groups [96, T] so per-head score matmuls slice at
  partition offsets {0,32,64} directly (offset 96 is unencodable).
- Scalar activation-table switches (Sqrt/Exp/Gelu) are prefetched with dummy
  ops so the ~1.5us ACT_TABLE_LOAD never sits on the critical path.
- LN chains (bn_stats/sqrt/recip/normalize/xbar) are emitted interleaved with
  the previous phase's residual adds, per 4-sub half, so the PE never waits
  on a cold LN chain at a phase boundary.
- Cross-cluster softmax leakage is killed by persistent zero off-diagonal
  blocks in P (normalize writes diagonal blocks only).
"""

import os
import numpy as np
import ml_dtypes

# ---- problem constants (hardcoded per contract) ----
B, N, D = 4, 16384, 192
HEADS, DH, CLM = 6, 32, 64
GRID_W = 128
DEPTH = 2
NCORES = 8
T = (B * N) // NCORES                # 8192 tokens per core
SUB = 128
NSUB = 8
TILE = SUB * NSUB                    # 1024-token supertile
NTILES = T // TILE                   # 8
DFF = 768

_STOP_PHASE = None   # debug: "qk"|"scores"|"attn"|None
_COMPILED = {}


def _scanline_order(pos, w):
    ix = np.floor(pos[..., 0]).astype(np.int64)
    iy = np.floor(pos[..., 1]).astype(np.int64)
    key = iy * w + np.where(iy % 2 == 1, w - 1 - ix, ix)
    return np.argsort(key, axis=1, kind="stable")


def _fold_weights(inputs):
    """Fold LN affine + biases into matmul weights. Returns per-layer dicts
    laid out exactly as the DRAM tensors the kernel declares."""
    bf16 = ml_dtypes.bfloat16
    scale = DH ** -0.5
    layers = []
    for i in range(DEPTH):
        g1 = np.asarray(inputs["ln1_g"][i], np.float64)
        b1 = np.asarray(inputs["ln1_b"][i], np.float64)
        Wqkv = np.asarray(inputs["w_qkv"][i], np.float64)
        bqkv = np.asarray(inputs["b_qkv"][i], np.float64)
        w_eff = g1[:, None] * Wqkv
        b_eff = b1 @ Wqkv + bqkv
        wq = w_eff[:, 0:D] * scale
        bq = b_eff[0:D] * scale
        wk = w_eff[:, D:2 * D]
        bk = b_eff[D:2 * D]
        wv = w_eff[:, 2 * D:3 * D]
        bv = b_eff[2 * D:3 * D]
        # qk weight M-layout: 3-head groups [q h0-2 | k h0-2 | q h3-5 | k h3-5]
        wqk = np.concatenate(
            [wq[:, :96], wk[:, :96], wq[:, 96:], wk[:, 96:]], axis=1)
        bqk = np.stack([bq[:96], bk[:96], bq[96:], bk[96:]], axis=1)
        wp = np.asarray(inputs["w_proj"][i], np.float64)
        bp = np.asarray(inputs["b_proj"][i], np.float64)
        g2 = np.asarray(inputs["ln2_g"][i], np.float64)
        b2 = np.asarray(inputs["ln2_b"][i], np.float64)
        W1 = np.asarray(inputs["w_fc1"][i], np.float64)
        w1_eff = g2[:, None] * W1
        b1_eff = b2 @ W1 + np.asarray(inputs["b_fc1"][i], np.float64)
        W2 = np.asarray(inputs["w_fc2"][i], np.float64)
        bfc2 = np.asarray(inputs["b_fc2"][i], np.float64)
        bv_t = np.stack([bv[:96], bv[96:]], axis=1)
        layers.append({
            f"wqk{i}": wqk.astype(bf16),
            f"bqk{i}": bqk.astype(np.float32),
            f"wv{i}": wv.astype(bf16),
            f"bv{i}": bv_t.astype(np.float32),
            f"wp{i}": wp.astype(bf16),
            f"bp{i}": np.tile(bp.astype(np.float32), (128, 1)),
            f"w1{i}": w1_eff.astype(bf16),
            f"b1{i}": b1_eff.reshape(6, 128).T.copy().astype(np.float32),
            f"w2{i}": W2.astype(bf16),
            f"b2{i}": np.tile(bfc2.astype(np.float32), (128, 1)),
        })
    return layers


def _build_nc(biases_zero=False, ntiles=NTILES):
    key = ("nc", biases_zero, ntiles)
    if key in _COMPILED:
        return _COMPILED[key]

    from contextlib import ExitStack
    import concourse.bass as bass
    import concourse.tile as tile
    from concourse import bacc, mybir
    from concourse.bass import ts, ds

    f32 = mybir.dt.float32
    bf16 = mybir.dt.bfloat16
    AF = mybir.ActivationFunctionType
    OP = mybir.AluOpType

    nc = bacc.Bacc("TRN2", target_bir_lowering=False, debug=False,
                   enable_asserts=False, num_devices=NCORES)

    x_d = nc.dram_tensor("x", [T, D], f32, kind="ExternalInput").ap()
    y_d = nc.dram_tensor("y", [T, D], f32, kind="ExternalOutput").ap()
    wd = []
    for i in range(DEPTH):
        wd.append({
            "wqk": nc.dram_tensor(f"wqk{i}", [D, 384], bf16, kind="ExternalInput").ap(),
            "bqk": nc.dram_tensor(f"bqk{i}", [96, 4], f32, kind="ExternalInput").ap(),
            "wv": nc.dram_tensor(f"wv{i}", [D, D], bf16, kind="ExternalInput").ap(),
            "bv": nc.dram_tensor(f"bv{i}", [96, 2], f32, kind="ExternalInput").ap(),
            "wp": nc.dram_tensor(f"wp{i}", [D, D], bf16, kind="ExternalInput").ap(),
            "bp": nc.dram_tensor(f"bp{i}", [128, D], f32, kind="ExternalInput").ap(),
            "w1": nc.dram_tensor(f"w1{i}", [D, DFF], bf16, kind="ExternalInput").ap(),
            "b1": nc.dram_tensor(f"b1{i}", [128, 6], f32, kind="ExternalInput").ap(),
            "w2": nc.dram_tensor(f"w2{i}", [DFF, D], bf16, kind="ExternalInput").ap(),
            "b2": nc.dram_tensor(f"b2{i}", [128, D], f32, kind="ExternalInput").ap(),
        })

    def bcast(ap2d, n):
        return bass.AP(tensor=ap2d.tensor, offset=ap2d.offset,
                       ap=[*ap2d.ap, [0, n]])

    with tile.TileContext(nc) as tc, ExitStack() as ctx:
        consts = ctx.enter_context(tc.tile_pool(name="consts", bufs=1))
        xpool = ctx.enter_context(tc.tile_pool(name="xpool", bufs=3))
        lnpool = ctx.enter_context(tc.tile_pool(name="lnpool", bufs=3))
        qkpool = ctx.enter_context(tc.tile_pool(name="qkpool", bufs=2))
        apool = ctx.enter_context(tc.tile_pool(name="apool", bufs=3))
        mpool = ctx.enter_context(tc.tile_pool(name="mpool", bufs=2))
        stpool = ctx.enter_context(tc.tile_pool(name="stpool", bufs=8))
        pp_sc = ctx.enter_context(tc.tile_pool(name="pp_sc", bufs=6, space="PSUM"))
        pp_x = ctx.enter_context(tc.tile_pool(name="pp_x", bufs=2, space="PSUM"))

        eps_t = consts.tile([128, 1], f32)
        nc.vector.memset(eps_t, 1e-5)
        u32 = mybir.dt.uint32
        rsq_c1 = consts.tile([128, 1], u32, name="rsq_c1")
        nc.vector.memset(rsq_c1, 1)
        rsq_cm = consts.tile([128, 1], u32, name="rsq_cm")
        nc.vector.memset(rsq_cm, 0x5F3759DF)
        dsrc = consts.tile([128, 1], f32)
        nc.vector.memset(dsrc, 0.5)
        ddst = consts.tile([128, 1], f32, name="ddst")

        def prefetch(af):
            nc.scalar.activation(ddst, dsrc, af)

        # persistent softmax tiles: off-diagonal (cross-cluster) blocks stay 0
        NPBUF = 4
        p_bufs = []
        for pb_i in range(NPBUF):
            pb = consts.tile([128, 2, HEADS, 128], bf16, name=f"pbuf{pb_i}")
            nc.vector.memset(pb, 0.0)
            p_bufs.append(pb)
        pctr = [0]

        # --- weights to SBUF once ---
        W = []
        for i in range(DEPTH):
            d = wd[i]
            sb = {}
            sb["wqkA"] = consts.tile([128, 384], bf16, name=f"wqkA{i}")
            sb["wqkB"] = consts.tile([128, 384], bf16, name=f"wqkB{i}")
            nc.sync.dma_start(out=sb["wqkA"], in_=d["wqk"][0:128])
            nc.sync.dma_start(out=sb["wqkB"][ds(64, 64)], in_=d["wqk"][128:192])
            sb["wvA"] = consts.tile([128, D], bf16, name=f"wvA{i}")
            sb["wvB"] = consts.tile([128, D], bf16, name=f"wvB{i}")
            nc.sync.dma_start(out=sb["wvA"], in_=d["wv"][0:128])
            nc.sync.dma_start(out=sb["wvB"][ds(64, 64)], in_=d["wv"][128:192])
            sb["wpA"] = consts.tile([96, D], bf16, name=f"wpA{i}")
            sb["wpB"] = consts.tile([96, D], bf16, name=f"wpB{i}")
            nc.sync.dma_start(out=sb["wpA"], in_=d["wp"][0:96])
            nc.sync.dma_start(out=sb["wpB"], in_=d["wp"][96:192])
            sb["w1A"] = consts.tile([128, DFF], bf16, name=f"w1A{i}")
            sb["w1B"] = consts.tile([128, DFF], bf16, name=f"w1B{i}")
            nc.sync.dma_start(out=sb["w1A"], in_=d["w1"][0:128])
            nc.sync.dma_start(out=sb["w1B"][ds(64, 64)], in_=d["w1"][128:192])
            sb["w2m"] = consts.tile([128, 6, D], bf16, name=f"w2m{i}")
            nc.sync.dma_start(
                out=sb["w2m"],
                in_=d["w2"].rearrange("(m p) n -> p m n", p=128))
            for nm, shp in (("bqk", [96, 4]), ("bv", [96, 2]), ("b1", [128, 6]),
                            ("bp", [128, D]), ("b2", [128, D])):
                sb[nm] = consts.tile(shp, f32, name=f"{nm}{i}")
                nc.sync.dma_start(out=sb[nm], in_=d[nm])
            W.append(sb)

        # ---------- LN chain helpers ----------
        def ln_tiles():
            return {
                "mv": lnpool.tile([128, NSUB, 2], f32, tag="mv", name="mv"),
                "sd": lnpool.tile([128, NSUB], f32, tag="sd", name="sd"),
                "r": lnpool.tile([128, NSUB], f32, tag="r", name="r"),
                "xnA": lnpool.tile([128, NSUB, 128], bf16, tag="xnA", name="xnA"),
                "xnB": lnpool.tile([128, NSUB, 128], bf16, tag="xnB", name="xnB"),
                "fmA": lnpool.tile([128, NSUB, 128], bf16, tag="fmA", name="fmA"),
                "fmB": lnpool.tile([128, NSUB, 128], bf16, tag="fmB", name="fmB"),
            }

        def ln_stats(ln, x_t, s):
            st = stpool.tile([128, 6], f32, tag="st", name="st")
            nc.vector.bn_stats(st, x_t[:, s, 0:D])
            nc.vector.bn_aggr(ln["mv"][:, s], st)

        def ln_finish_half(ln, x_t, h):
            # r = 1/sqrt(var) on DVE: magic-constant seed + 2 Newton steps
            # (~5e-6 rel err; the 1e-5 eps is dropped, negligible at var~1).
            # Keeps Sqrt off the scalar engine so the Exp/Gelu activation
            # tables never get evicted mid-phase.
            s0 = 4 * h
            var = ln["mv"][:, s0:s0 + 4, 1]
            t1 = ln["sd"][:, s0:s0 + 4]
            yv = stpool.tile([128, 4], f32, tag="ya", name="ya")
            av = stpool.tile([128, 4], f32, tag="av", name="av")
            rr4 = ln["r"][:, s0:s0 + 4]
            nc.vector.tensor_tensor(out=t1.bitcast(u32), in0=var.bitcast(u32),
                                    in1=bcast(rsq_c1, 4), op=OP.logical_shift_right)
            nc.vector.tensor_tensor(out=yv.bitcast(u32), in0=bcast(rsq_cm, 4),
                                    in1=t1.bitcast(u32), op=OP.subtract)
            cur = yv
            for it in range(2):
                nxt = rr4 if it == 1 else t1
                nc.vector.tensor_tensor(out=av, in0=var, in1=cur, op=OP.mult)
                nc.vector.tensor_tensor(out=av, in0=av, in1=cur, op=OP.mult)
                nc.vector.tensor_scalar(out=av, in0=av, scalar1=-0.5,
                                        scalar2=1.5, op0=OP.mult, op1=OP.add)
                nc.vector.tensor_tensor(out=nxt, in0=cur, in1=av, op=OP.mult)
                cur = nxt
            for s in range(s0, s0 + 4):
                mu = ln["mv"][:, s, 0:1]
                rr = ln["r"][:, s:s + 1]
                mu_b = bcast(mu, 128)
                r_b = bcast(rr, 128)
                tA = stpool.tile([128, 128], bf16, tag="tA", name="tA")
                nc.gpsimd.tensor_tensor(out=tA, in0=x_t[:, s, 0:128],
                                        in1=mu_b, op=OP.subtract)
                nc.gpsimd.tensor_tensor(out=ln["xnA"][:, s], in0=tA,
                                        in1=r_b, op=OP.mult)
                tB = stpool.tile([128, 128], bf16, tag="tB", name="tB")
                eng = nc.vector if s % 2 == 0 else nc.gpsimd
                eng.tensor_tensor(out=tB, in0=x_t[:, s, 64:192],
                                  in1=mu_b, op=OP.subtract)
                eng.tensor_tensor(out=ln["xnB"][:, s], in0=tB,
                                  in1=r_b, op=OP.mult)
            nc.sync.dma_start_transpose(out=ln["fmA"][:, s0:s0 + 4, :],
                                        in_=ln["xnA"][:, s0:s0 + 4, :])
            nc.sync.dma_start_transpose(out=ln["fmB"][:, s0:s0 + 4, :],
                                        in_=ln["xnB"][:, s0:s0 + 4, :])

        def fmA_half(ln, h):
            return ln["fmA"][:, 4 * h:4 * h + 4, :].rearrange("p a b -> p (a b)")

        def fmB_half(ln, h):
            return ln["fmB"][ds(64, 64), 4 * h:4 * h + 4, :].rearrange(
                "p a b -> p (a b)")

        # ======================================================================
        def emit_layer(sb, x_t, ln1, x_nxt, nxt_is_tile):
            """Emit one transformer layer. ln1 holds this layer's LN1 (already
            emitted). Returns the LN tiles for the NEXT layer's LN1 (computed
            on x_nxt: either this x_t post-MLP, or the next tile's x)."""
            # ---- qk: 4 blocks x 2 halves ----
            qA = qkpool.tile([96, TILE], bf16, tag="qA", name="qA")
            kA = qkpool.tile([96, TILE], bf16, tag="kA", name="kA")
            qB = qkpool.tile([96, TILE], bf16, tag="qB", name="qB")
            kB = qkpool.tile([96, TILE], bf16, tag="kB", name="kB")
            dsts = (qA, kA, qB, kB)
            for hf in range(2):
                rA, rB = fmA_half(ln1, hf), fmB_half(ln1, hf)
                for m in range(4):
                    ps = pp_x.tile([96, 512], f32, tag="x", name="psqk")
                    nc.tensor.matmul(ps, sb["wqkA"][:, ds(96 * m, 96)], rA,
                                     start=True, stop=False)
                    nc.tensor.matmul(ps, sb["wqkB"][ds(64, 64), ds(96 * m, 96)], rB,
                                     start=False, stop=True)
                    dst = dsts[m][:, ts(hf, 512)]
                    if not biases_zero:
                        nc.scalar.activation(dst, ps, AF.Identity,
                                             bias=sb["bqk"][:, m:m + 1])
                    else:
                        nc.vector.tensor_copy(dst, ps)

            if _STOP_PHASE == "qk":
                return ln1
            # ---- scores + softmax + v, per sub ----
            v_tm = apool.tile([128, NSUB, D], bf16, tag="vtm", name="v_tm")
            pkms = []
            for s in range(NSUB):
                # one PSUM tile per PE tile-row: heads (hh, hh+3) share row
                # 32*hh; independent accumulation groups at different rows in
                # one PSUM tile lock up the device
                scR = [pp_sc.tile([128, 2, 128], f32, tag="sc", name=f"scR{r}")
                       for r in range(3)]
                for hd in range(HEADS):
                    grp, hh = hd // 3, hd % 3
                    qsrc = (qA, qB)[grp]
                    ksrc = (kA, kB)[grp]
                    nc.tensor.matmul(scR[hh][:, grp],
                                     qsrc[ds(32 * hh, 32), ts(s, 128)],
                                     ksrc[ds(32 * hh, 32), ts(s, 128)],
                                     start=True, stop=True)
                if _STOP_PHASE == "sconly":
                    continue
                # v for this sub (PE filler between score groups)
                psv = pp_x.tile([128, D], f32, tag="x", name="psv")
                nc.tensor.matmul(psv, ln1["fmA"][:, s, :], sb["wvA"],
                                 start=True, stop=False)
                nc.tensor.matmul(psv, ln1["fmB"][ds(64, 64), s, :], sb["wvB"][ds(64, 64)],
                                 start=False, stop=True)
                nc.vector.tensor_copy(v_tm[:, s], psv)
                if _STOP_PHASE == "sv":
                    continue
                # softmax (diagonal 64-blocks only)
                E = apool.tile([128, HEADS, 128], bf16, tag="E", name="E")
                sums = stpool.tile([128, HEADS], bf16, tag="sm", name="sums")
                rsum = stpool.tile([128, HEADS], f32, tag="rs", name="rsum")
                for hh in range(3):
                    esl = bass.AP(
                        tensor=E.tensor,
                        offset=E.offset + hh * 128,
                        ap=[E.ap[0], [3 * 128, 2], [1, 128]])
                    nc.scalar.activation(esl, scR[hh], AF.Exp)
                if _STOP_PHASE == "exp":
                    continue
                with nc.allow_low_precision(reason="softmax sums bf16, "
                                             "0.4% well under tolerance"):
                    nc.vector.reduce_sum(sums[0:64], E[0:64, :, 0:64],
                                         axis=mybir.AxisListType.X)
                    nc.vector.reduce_sum(sums[ds(64, 64)],
                                         E[ds(64, 64), :, 64:128],
                                         axis=mybir.AxisListType.X)
                nc.vector.reciprocal(rsum, sums)
                if _STOP_PHASE == "sums":
                    continue
                P = p_bufs[(pctr[0] // 2) % NPBUF]
                sp = pctr[0] % 2
                pctr[0] += 1
                for half in range(2):
                    hs = ds(half * 64, 64)
                    rs_half = rsum[ds(half * 64, 64)]
                    nc.gpsimd.tensor_tensor(
                        out=P[hs, sp, :, hs], in0=E[hs, :, hs],
                        in1=bcast(rs_half, 64), op=OP.mult)
                if _STOP_PHASE == "norm":
                    continue
                if sp == 1:
                    pkm = apool.tile([128, 2 * HEADS, 128], bf16, tag="pkm",
                                     name="pkm")
                    nc.sync.dma_start_transpose(
                        out=pkm, in_=P.rearrange("p a b c -> p (a b c)"))
                    pkms.append(pkm)

            if _STOP_PHASE in ("sconly", "sconly2", "sv", "exp", "sums", "norm", "scores"):
                return ln1
            # next-tile LN1 depends only on the next x DMA: emit its whole
            # chain here so it overlaps this layer's attention tail + MLP
            ln_early = None
            if nxt_is_tile and x_nxt is not None:
                ln_early = ln_tiles()
                for s2 in range(NSUB):
                    ln_stats(ln_early, x_nxt, s2)
                ln_finish_half(ln_early, x_nxt, 0)
                ln_finish_half(ln_early, x_nxt, 1)

            # ---- O + out-copies + proj + residual + LN2 stats, per sub ----
            ln2 = ln_tiles()
            ofm0 = apool.tile([96, TILE], bf16, tag="ofm0", name="ofm0")
            ofm1 = apool.tile([96, TILE], bf16, tag="ofm1", name="ofm1")
            for s in range(NSUB):
                oP = pp_x.tile([96, 256], f32, tag="x", name="oP")
                for hd in range(HEADS):
                    grp, hh = hd // 3, hd % 3
                    nc.tensor.matmul(oP[ds(32 * hh, 32), ts(grp, 128)],
                                     v_tm[:, s, ts(hd, 32)],
                                     pkms[s // 2][:, 6 * (s % 2) + hd],
                                     start=True, stop=True,
                                     tile_position=(0, 32 * hh))
                if not biases_zero:
                    nc.scalar.activation(ofm0[:, ts(s, 128)], oP[:, 0:128],
                                         AF.Identity, bias=sb["bv"][:, 0:1])
                    nc.scalar.activation(ofm1[:, ts(s, 128)], oP[:, 128:256],
                                         AF.Identity, bias=sb["bv"][:, 1:2])
                else:
                    nc.vector.tensor_copy(ofm0[:, ts(s, 128)], oP[:, 0:128])
                    nc.vector.tensor_copy(ofm1[:, ts(s, 128)], oP[:, 128:256])
                psp = pp_x.tile([128, D], f32, tag="x", name="psp")
                nc.tensor.matmul(psp, ofm0[:, ts(s, 128)], sb["wpA"],
                                 start=True, stop=False)
                nc.tensor.matmul(psp, ofm1[:, ts(s, 128)], sb["wpB"],
                                 start=False, stop=True)
                nc.vector.tensor_add(x_t[:, s, 0:D], x_t[:, s, 0:D], psp)
                if not biases_zero:
                    nc.vector.tensor_add(x_t[:, s, 0:D], x_t[:, s, 0:D],
                                         sb["bp"])
                ln_stats(ln2, x_t, s)
                if s == 3:
                    ln_finish_half(ln2, x_t, 0)
                elif s == 7:
                    ln_finish_half(ln2, x_t, 1)
                    prefetch(AF.Gelu_apprx_tanh)

            if _STOP_PHASE == "attn":
                return ln2
            # ---- MLP: fc1+gelu both halves, then fc2 + residual ----
            hfms = []
            for hf in range(2):
                rA, rB = fmA_half(ln2, hf), fmB_half(ln2, hf)
                hfm = mpool.tile([128, 6, 512], bf16, tag="hfm", name="hfm")
                for m in range(6):
                    ps = pp_x.tile([128, 512], f32, tag="x", name="psh")
                    nc.tensor.matmul(ps, sb["w1A"][:, ts(m, 128)], rA,
                                     start=True, stop=False)
                    nc.tensor.matmul(ps, sb["w1B"][ds(64, 64), ts(m, 128)], rB,
                                     start=False, stop=True)
                    if biases_zero:
                        nc.scalar.activation(hfm[:, m], ps, AF.Gelu_apprx_tanh)
                    else:
                        nc.scalar.activation(hfm[:, m], ps, AF.Gelu_apprx_tanh,
                                             bias=sb["b1"][:, m:m + 1])
                hfms.append(hfm)
            ln_n = ln_early if (nxt_is_tile and ln_early is not None) else ln_tiles()
            for hf in range(2):
                for si in range(4):
                    s = 4 * hf + si
                    ps = pp_x.tile([128, D], f32, tag="x", name="psf2")
                    for m in range(6):
                        nc.tensor.matmul(ps, hfms[hf][:, m, ts(si, 128)],
                                         sb["w2m"][:, m],
                                         start=(m == 0), stop=(m == 5))
                    nc.vector.tensor_add(x_t[:, s, 0:D], x_t[:, s, 0:D], ps)
                    if not biases_zero:
                        nc.vector.tensor_add(x_t[:, s, 0:D], x_t[:, s, 0:D],
                                             sb["b2"])
                    if x_nxt is None or nxt_is_tile:
                        continue
                    ln_stats(ln_n, x_t, s)
                if x_nxt is None or nxt_is_tile:
                    continue
                if hf == 0:
                    ln_finish_half(ln_n, x_t, 0)
                else:
                    ln_finish_half(ln_n, x_t, 1)
                    prefetch(AF.Exp)
            if nxt_is_tile:
                prefetch(AF.Exp)
            return ln_n

        # ======================================================================
        def load_tile(it):
            xt = xpool.tile([128, NSUB, D], f32, tag="x", name="x_t")
            nc.sync.dma_start(
                out=xt, in_=x_d[ts(it, TILE)].rearrange("(s p) f -> p s f",
                                                        p=128))
            return xt

        x_cur = load_tile(0)
        ln = ln_tiles()
        for s in range(4):
            ln_stats(ln, x_cur, s)
        ln_finish_half(ln, x_cur, 0)
        for s in range(4, 8):
            ln_stats(ln, x_cur, s)
        ln_finish_half(ln, x_cur, 1)
        prefetch(AF.Exp)

        for it in range(ntiles):
            x_nxt = load_tile(it + 1) if it + 1 < ntiles else None
            # layer 0: next LN is layer 1's LN1 on this x_t
            ln = emit_layer(W[0], x_cur, ln, x_cur, nxt_is_tile=False)
            # layer 1: next LN is the next tile's LN1 on x_nxt
            ln = emit_layer(W[1], x_cur, ln, x_nxt, nxt_is_tile=True)
            nc.sync.dma_start(
                out=y_d[ts(it, TILE)].rearrange("(s p) f -> p s f", p=128),
                in_=x_cur[:, :, 0:D])
            x_cur = x_nxt

    nc.compile()
    _COMPILED[key] = nc
    return nc


def _ensure_ntff_hook():
    """The image's antenv package lacks axon_hooks; synthesize it and install
    the ctypes-based NTFF profile hook from trn_agent_boot (test-only path)."""
    import sys, types
    if "antenv.axon_hooks" in sys.modules:
        return True
    try:
        mod = types.ModuleType("antenv.axon_hooks")
        state = {}
        mod.set_axon_ntff_profile_hook = lambda h: state.__setitem__("h", h)
        mod.get_axon_ntff_profile_hook = lambda: state.get("h")
        sys.modules["antenv.axon_hooks"] = mod
        import antenv
        antenv.axon_hooks = mod
        from trn_agent_boot.trn_boot import _ntff_profile_via_ctypes
        mod.set_axon_ntff_profile_hook(
            _ntff_profile_via_ctypes("/opt/axon/libaxon_pjrt.so"))
        return True
    except Exception as e:  # pragma: no cover
        print(f"NTFF hook shim failed: {e}")
        return False


def _run(inputs, trace=False):
    """Shard, execute on 8 cores, gather. Returns (y_full, exec_time_ns)."""
    from concourse.bass_utils import run_bass_kernel_spmd

    if trace:
        trace = _ensure_ntff_hook()

    layers = _fold_weights(inputs)
    bz = all(
        not np.any(np.asarray(d[k], np.float32))
        for d in layers for k in d
        if k.startswith(("bp", "b2", "bqk", "bv", "b1")))
    nc = _build_nc(biases_zero=bz)

    x = np.asarray(inputs["x"], np.float32)
    pos = np.asarray(inputs["pos"], np.float32)
    w = int(np.asarray(inputs["w"]))
    order = _scanline_order(pos, w)
    x_ord = np.take_along_axis(x, order[..., None], axis=1)
    shards = np.ascontiguousarray(x_ord.reshape(NCORES, T, D))

    wmap = {}
    for d in layers:
        wmap.update({k: np.ascontiguousarray(v) for k, v in d.items()})

    in_maps = [{"x": shards[c], **wmap} for c in range(NCORES)]
    res = run_bass_kernel_spmd(nc, in_maps, core_ids=list(range(NCORES)),
                               trace=trace)
    y_ord = np.stack([res.results[c]["y"] for c in range(NCORES)])
    y_ord = y_ord.reshape(B, N, D)
    y = np.empty_like(y_ord)
    np.put_along_axis(y, order[..., None], y_ord, axis=1)
    return y.astype(np.float32), res.exec_time_ns


def kernel(**inputs):
    y, _ = _run(inputs, trace=False)
    return y
